# revision 1
# baseline (speedup 1.0000x reference)
"""LuminanceLoss Bass kernel for 8 TRN2 NeuronCores.

Reference: loss = mean(|L(gen) - L(tgt)|) with L = CIE-Lab L channel of
sRGB images in [-1,1], shape (64,3,512,512) f32.

Math per element x (per channel c, weight w_c):
    s   = (x+1)/2
    lin = where(s > 0.04045, ((s+0.055)/1.055)^2.4, s/12.92)
    Y   = sum_c w_c * lin_c
    f   = where(Y > eps, Y^(1/3), kappa*Y + 16/116)
    |L_g - L_t| = 116 * |f_g - f_t|

Mapping to engines (per core: 8 images x 2 tensors, standard ops only —
this container's walrus rejects custom-DVE instructions):
  ACT: t = Ln(a*x+b); e_c = Exp(2.4*t + ln(w_c))  == w_c*((s+.055)/1.055)^2.4
       yl = Ln(max(Y,eps)); cb = Exp(yl/3)        == cbrt(max(Y,eps))
  DVE: l_c = x*(w_c*m0) + (w_c*m0)                (tensor_scalar affine)
       lin_c = max(e_c, l_c)    piecewise select via max: the two sRGB
           branches cross at s=0.04045, so max is exact for s>0.0393;
           below that the error is <=8e-4 on ~2% of elements (validated:
           total rel err 3.3e-4 incl. bf16 intermediates)
       f = min(cb, kappa*Y+16/116)   EXACT: the linear segment is the
           tangent of the (concave) cbrt at eps, so min() selects the
           reference branch everywhere once cbrt's input is clamped
       sum_fd |f_g - f_t| via tensor_reduce(add, abs=True)
  Intermediates in bf16; per-partition sums accumulate in f32.

Sharding: batch dim 64 -> 8 cores x 8 images (pure data parallel).
Each core returns a [128,1] f32 partial-sum vector; host sums and scales
by 116/N (the -16 offsets of L cancel in the difference).

Measured (cost-model TimelineSim + HW slope via in-NEFF repetition):
~235-265 us/core vs ~140 us memory roofline (ACT-bound: 96 activation
instructions ~264 us busy; DVE ~242 us; DMA engines ~140 us).
"""

import numpy as np

import concourse.bass as bass
import concourse.mybir as mybir
from concourse.bass_utils import run_bass_kernel_spmd
from concourse.tile import TileContext

# ----------------------------------------------------------------- patch
# The walrus build in this container rejects instructions whose sync_info
# carries >2 waits ("Too many sync wait commands", CoreV3GenImpl.cpp:104)
# — the Tile kernel-tail Drain aggregates one wait per live proc.  Split
# that single multi-wait Drain into a chain of single-wait drains on the
# sync queue (executed serially -> semantically identical).
_ORIG_DRAIN_AND_BARRIER = TileContext._drain_and_barrier


def _patched_drain_and_barrier(self, tick_clock, wait_clock):
    from concourse.vector_clock import ScopedClock

    drain_inst = self.nc.sync.drain()
    wait_clock.add_sem_waits(
        drain_inst.ins, ScopedClock({None: tick_clock.global_clock})
    )
    si = drain_inst.ins.sync_info
    if si is not None and len(si.on_wait) > 1:
        waits = list(si.on_wait)
        drain_inst.ins.sync_info = mybir.SyncInfo(
            on_wait=waits[:1], on_update=list(si.on_update)
        )
        for w in waits[1:]:
            extra = self.nc.sync.drain()
            extra.ins.sync_info = mybir.SyncInfo(on_wait=[w], on_update=[])

    self.nc.all_engine_barrier()
    assert self.sems is not None
    popped = self.nc._tile_sem_poison_stack.pop()
    assert popped is self._sem_poison
    self.nc.clear_and_free_semaphores(list(self.sems.allocated().values()))
    self.nc.all_engine_barrier()


TileContext._drain_and_barrier = _patched_drain_and_barrier


def _split_excess_waits(nc, max_waits=1):
    """Walrus here rejects any instruction with >1 sem wait.  Move extra
    waits onto preceding NoOps on the same engine stream (streams execute
    in order, so waiting on the NoOps then the instruction is identical)."""
    for fn in nc.m.functions:
        for bb in fn.blocks:
            new = []
            for inst in bb.instructions:
                si = getattr(inst, "sync_info", None)
                if si is not None and len(si.on_wait) > max_waits:
                    waits = list(si.on_wait)
                    for w in waits[max_waits:]:
                        nop = mybir.InstNoOp(
                            name=nc.get_next_instruction_name(),
                            engine=inst.engine,
                            sync_info=mybir.SyncInfo(on_wait=[w], on_update=[]),
                            bass_nofuse=True,
                        )
                        nc.register_instruction(nop, overwrite=True)
                        new.append(nop)
                    inst.sync_info = mybir.SyncInfo(
                        on_wait=waits[:max_waits], on_update=list(si.on_update)
                    )
                new.append(inst)
            bb.instructions[:] = new

# ---------------------------------------------------------------- constants
P = 128          # SBUF partitions
F = 2048         # free-dim elements per 512x512 plane per partition
IMGS = 8         # images per core
N_CORES = 8
N_TOTAL = 64 * 512 * 512

A_ = 0.5 / 1.055                 # s=(x+1)/2 ; u=(s+.055)/1.055 = A_*x + B_
B_ = 0.555 / 1.055
THX = 2.0 * 0.04045 - 1.0        # s > 0.04045  <=>  x > THX
M0 = 0.5 / 12.92                 # s/12.92 = M0*x + M0
W = (0.2126729, 0.7151522, 0.0721750)
EPS = 0.008856
KAPPA = 7.787
C16 = 16.0 / 116.0

F32 = mybir.dt.float32
BF16 = mybir.dt.bfloat16
Ln = mybir.ActivationFunctionType.Ln
Exp = mybir.ActivationFunctionType.Exp

# ------------------------------------------------------------- program
_NC_CACHE = {}


def _build_program(reps=1):
    if reps in _NC_CACHE:
        return _NC_CACHE[reps]
    import math

    nc = bass.Bass()
    # const APs for activation biases (bias must be a [P,1] AP for non-Copy)
    for val in (B_, math.log(W[0]), math.log(W[1]), math.log(W[2])):
        v = float(np.float32(val))
        t_ = nc.alloc_sbuf_tensor(f"const-b-{v}", [P, 1], F32)
        nc.gpsimd.memset(t_.ap(), v)
        nc.const_aps.aps[(F32, v)] = t_.ap()
    nc.all_engine_barrier()

    gen = nc.dram_tensor("generated", [IMGS, 3, 512, 512], F32, kind="ExternalInput")
    tgt = nc.dram_tensor("target", [IMGS, 3, 512, 512], F32, kind="ExternalInput")
    out = nc.dram_tensor("out", [P, 1], F32, kind="ExternalOutput")

    AOT = mybir.AluOpType

    with TileContext(nc) as tc:
        with (
            tc.tile_pool(name="x", bufs=2) as xp,
            tc.tile_pool(name="tln", bufs=1) as tp,
            tc.tile_pool(name="e", bufs=2) as ep,
            tc.tile_pool(name="lw", bufs=1) as lwp,
            tc.tile_pool(name="lin", bufs=1) as lp,
            tc.tile_pool(name="y", bufs=1) as yp,
            tc.tile_pool(name="ym", bufs=2) as ymp,
            tc.tile_pool(name="ylog", bufs=1) as ylp,
            tc.tile_pool(name="cb", bufs=2) as cbp,
            tc.tile_pool(name="f", bufs=4) as fp,
            tc.tile_pool(name="misc", bufs=1) as mp,
        ):
            acc = mp.tile([P, IMGS * reps], F32, tag="acc")
            for it in range(IMGS * reps):
                img = it % IMGS
                f_pair = []
                for src in (gen, tgt):
                    x = xp.tile([P, 3, F], F32, tag="x")
                    nc.sync.dma_start(
                        out=x[:],
                        in_=src[img].rearrange("c (p r) w -> p c (r w)", p=P, r=4),
                    )
                    # t = ln((s+.055)/1.055), s=(x+1)/2   [one ACT op, FD=6144]
                    tl = tp.tile([P, 3, F], BF16, tag="tln")
                    nc.scalar.activation(
                        tl[:], x[:], Ln,
                        bias=float(np.float32(B_)), scale=float(np.float32(A_)),
                    )
                    # e_c = w_c * u^2.4 = Exp(2.4*t + ln(w_c))  -> bf16
                    e = ep.tile([P, 3, F], BF16, tag="e")
                    for c in range(3):
                        nc.scalar.activation(
                            e[:, c], tl[:, c], Exp,
                            bias=float(np.float32(math.log(W[c]))), scale=2.4,
                        )
                    # l_c = w_c*(s/12.92) = x*(w_c*m0) + (w_c*m0)   -> bf16
                    lw = lwp.tile([P, 3, F], BF16, tag="lw")
                    for c in range(3):
                        wm = float(np.float32(W[c] * M0))
                        nc.vector.tensor_scalar(
                            out=lw[:, c], in0=x[:, c],
                            scalar1=wm, scalar2=wm,
                            op0=AOT.mult, op1=AOT.add,
                        )
                    # lin_c = max(e_c, l_c)  (exact except s<0.0393: rel err ~3e-4)
                    lin = lp.tile([P, 3, F], BF16, tag="lin")
                    for c in range(3):
                        nc.vector.tensor_tensor(
                            out=lin[:, c], in0=e[:, c], in1=lw[:, c], op=AOT.max
                        )
                    # Y = lin_R + lin_G + lin_B
                    y = yp.tile([P, F], BF16, tag="y")
                    nc.vector.tensor_add(out=y[:], in0=lin[:, 0], in1=lin[:, 1])
                    nc.vector.tensor_add(out=y[:], in0=y[:], in1=lin[:, 2])
                    # f = min(cbrt(max(Y,eps)), kappa*Y + 16/116)   (exact)
                    ym = ymp.tile([P, F], BF16, tag="ym")
                    nc.vector.tensor_scalar_max(
                        out=ym[:], in0=y[:], scalar1=float(np.float32(EPS))
                    )
                    yl = ylp.tile([P, F], F32, tag="ylog")
                    nc.scalar.activation(yl[:], ym[:], Ln)
                    cb = cbp.tile([P, F], BF16, tag="cb")
                    nc.scalar.activation(cb[:], yl[:], Exp, scale=1.0 / 3.0)
                    tg = yp.tile([P, F], BF16, tag="tang")
                    nc.vector.tensor_scalar(
                        out=tg[:], in0=y[:], scalar1=float(np.float32(KAPPA)),
                        scalar2=float(np.float32(C16)), op0=AOT.mult, op1=AOT.add,
                    )
                    f = fp.tile([P, F], BF16, tag="f")
                    nc.vector.tensor_tensor(
                        out=f[:], in0=cb[:], in1=tg[:], op=AOT.min
                    )
                    f_pair.append(f)
                # acc[:, img] = sum_fd |f_g - f_t|
                d = yp.tile([P, F], BF16, tag="d")
                nc.vector.tensor_sub(out=d[:], in0=f_pair[0][:], in1=f_pair[1][:])
                nc.vector.tensor_reduce(
                    out=acc[:, it : it + 1], in_=d[:],
                    axis=mybir.AxisListType.X, op=AOT.add,
                    apply_absolute_value=True,
                )
            tot = mp.tile([P, 1], F32, tag="tot")
            nc.vector.reduce_sum(out=tot[:], in_=acc[:], axis=mybir.AxisListType.X)
            nc.sync.dma_start(out=out[:], in_=tot[:])

    _split_excess_waits(nc)
    _NC_CACHE[reps] = nc
    return nc


# --------------------------------------------------------------- entry
def _run(inputs, **spmd_kwargs):
    nc = _build_program()
    g = np.ascontiguousarray(np.asarray(inputs["generated"], dtype=np.float32))
    t = np.ascontiguousarray(np.asarray(inputs["target"], dtype=np.float32))
    assert g.shape == (64, 3, 512, 512) and t.shape == (64, 3, 512, 512)
    in_maps = [
        {
            "generated": np.ascontiguousarray(g[i * IMGS : (i + 1) * IMGS]),
            "target": np.ascontiguousarray(t[i * IMGS : (i + 1) * IMGS]),
        }
        for i in range(N_CORES)
    ]
    res = run_bass_kernel_spmd(nc, in_maps, list(range(N_CORES)), **spmd_kwargs)
    total = float(
        sum(np.asarray(r["out"], np.float64).sum() for r in res.results)
    )
    loss = np.float32(116.0 * total / N_TOTAL)
    return np.asarray(loss, dtype=np.float32), res


def kernel(generated, target):
    out, _ = _run({"generated": generated, "target": target})
    return out

